# revision 46
# baseline (speedup 1.0000x reference)
"""Trainium2 Bass kernel for nn_AttentionLayer (self-attention over 64x64 images).

Computation (per batch image b):
    xf = x[b].reshape(C, N)                     # C=256, N=4096
    q = BN(Wq @ xf + bq)   -> [32, N]
    k = BN(Wk @ xf + bk)   -> [32, N]
    v = BN(Wv @ xf + bv)   -> [256, N]
    attn = softmax_j(q^T k) -> [N, N]
    out = v @ attn^T        -> [256, N]
    y = gamma * out + xf

Sharding: 8 cores = 4 batches x 2 query-row halves; no collectives.

Host-side algebra folds (all exact):
  - BN folded into weights/bias.
  - k bias drops entirely (adds a per-query constant to every logit row ->
    cancels in softmax); q bias kept (its logit term varies over keys).
  - v bias passes through softmax unchanged (rows sum to 1), so
    y = (gamma*vhat @ P)*recip + (x + gamma*bv); gamma folds into Wv and the
    residual bias rb = gamma*bv is added per-channel in the epilogue.
  - Each core's xbh has its own query-half columns FIRST (keys are
    order-invariant), so the query slice is a view of the key tensor and the
    bf16 x tile doubles as the residual.

Device (per core, all matmuls bf16 in / f32 psum):
  - q/k proj: 2 col-group replica matmuls -> [64, N*]; q evicted via ACT
    Identity+bias, k via DVE copy; DVE copy replicates to partitions 64-127
    so S^T rounds can run 4 concurrent row-band (32x128-tile) matmuls.
  - V^T[j, c] = x^T Wv^T computed directly in [j, c] layout.
  - Main loop over 4 i-blocks x 16 rounds of 2 key-chunks: S^T row-band
    matmul pair -> exp on ScalarE (no max subtraction; |S|<60 fits f32/bf16)
    -> P^T bf16 -> out psum accumulation + bf16 rowsum chains on DVE.
    S matmuls for round pairs are emitted adjacently for 4-way PE tile
    concurrency and fewer tile-mode switches.
  - Epilogue per i-block: outp evicted to SBUF (frees psum banks), rowsum
    via ones-matmuls, reciprocal on DVE, PE rank-1 broadcast, fused
    (out*recip + rb) + x on DVE, DMA out.
"""

import numpy as np
from contextlib import ExitStack

import ml_dtypes
import concourse.bass as bass
import concourse.mybir as mybir
import concourse.tile as tile
from concourse import bacc
from concourse.bass_utils import run_bass_kernel_spmd

B, C, H, W = 4, 256, 64, 64
N = H * W            # 4096 tokens per image
CQ = C // 8          # 32 q/k channels
NQ = N // 2          # 2048 query tokens per core
EPS = 1e-5
P = 128
IB = 512             # i-block (psum bank of f32)
NJC = N // P         # 32 j-chunks
NSR = NJC // 2       # 16 rounds per i-block (2 j-chunks each)
NCORES = 8

f32 = mybir.dt.float32
bf16 = mybir.dt.bfloat16
FT = mybir.ActivationFunctionType
ALU = mybir.AluOpType
BF = ml_dtypes.bfloat16

_CACHE = {}


def _build():
    nc = bacc.Bacc("TRN2", target_bir_lowering=False, debug=False,
                   num_devices=NCORES)
    xbh = nc.dram_tensor("xbh", [C, N], bf16, kind="ExternalInput").ap()
    wqT = nc.dram_tensor("wqT", [C, CQ], bf16, kind="ExternalInput").ap()
    wkT = nc.dram_tensor("wkT", [C, CQ], bf16, kind="ExternalInput").ap()
    wvT = nc.dram_tensor("wvT", [C, C], bf16, kind="ExternalInput").ap()
    bq2 = nc.dram_tensor("bq2", [2 * CQ, 1], f32, kind="ExternalInput").ap()
    rbh = nc.dram_tensor("rbh", [P, 2], f32, kind="ExternalInput").ap()
    y = nc.dram_tensor("y", [C, NQ], f32, kind="ExternalOutput").ap()

    with tile.TileContext(nc) as tc, ExitStack() as ctx:
        const = ctx.enter_context(tc.tile_pool(name="const", bufs=1))
        ones_col = const.tile([P, 1], bf16)
        nc.vector.memset(ones_col[:], 1.0)
        ones_row = const.tile([1, P], f32)
        nc.vector.memset(ones_row[:], 1.0)

        wq_sb = const.tile([P, 2, CQ], bf16)
        wk_sb = const.tile([P, 2, CQ], bf16)
        wv_sb = const.tile([P, 2, C], bf16)
        bq_sb = const.tile([2 * CQ, 1], f32)
        rb_sb = const.tile([P, 2], f32)

        garb = const.tile([P, 64], bf16)
        nc.vector.memset(garb[:], 0.5)

        xpool = ctx.enter_context(tc.tile_pool(name="x", bufs=1))
        xb_sb = [xpool.tile([P, N], bf16, name=f"xbsb{cc}") for cc in range(2)]

        # PE warmup during the input-DMA window: ~50 dependency-free tiny
        # matmuls keep the PE busy past the HAM activity window so the
        # projection phase starts at the full 2.4 GHz clock.
        with tc.tile_pool(name="warm_ps", bufs=1, space="PSUM") as wps:
            wtile = wps.tile([1, 64], f32, name="warm")
            for _ in range(72):
                nc.tensor.matmul(wtile[:], lhsT=ones_col[:], rhs=garb[:],
                                 start=True, stop=True)

        qkv = ctx.enter_context(tc.tile_pool(name="qkv", bufs=1))
        qrep = qkv.tile([2 * CQ, NQ], bf16)  # 2 row-band replicas of q
        krep = qkv.tile([2 * CQ, N], bf16)   # 2 row-band replicas of k
        vt_sb = qkv.tile([P, NJC, C], bf16)  # V^T as [j-in-chunk, jc, c]

        # ---- input DMA (query-half columns of xbh come first) ----
        nc.sync.dma_start(out=wq_sb[:], in_=wqT.rearrange("(k p) m -> p k m", k=2))
        nc.sync.dma_start(out=bq_sb[:], in_=bq2[:])
        for cc in range(2):   # first 512 columns alone so q proj starts early
            nc.sync.dma_start(out=xb_sb[cc][:, 0:IB],
                              in_=xbh[cc * P:(cc + 1) * P, 0:IB])
        for cc in range(2):
            nc.sync.dma_start(out=xb_sb[cc][:, IB:1024],
                              in_=xbh[cc * P:(cc + 1) * P, IB:1024])
        for cc in range(2):
            nc.sync.dma_start(out=xb_sb[cc][:, 1024:2048],
                              in_=xbh[cc * P:(cc + 1) * P, 1024:2048])
        nc.sync.dma_start(out=wk_sb[:],
                          in_=wkT.rearrange("(k p) m -> p k m", k=2))
        # second half of the keys + v-path weights issue from the Activation
        # queue (also a HWDGE) in parallel with the Sync queue's stream:
        # each dma_start costs ~0.6us of issue time on its queue.
        for cc in range(2):
            nc.scalar.dma_start(out=xb_sb[cc][:, 2048:N],
                                in_=xbh[cc * P:(cc + 1) * P, 2048:N])
        nc.scalar.dma_start(out=wv_sb[:],
                            in_=wvT.rearrange("(k p) m -> p k m", k=2))
        nc.sync.dma_start(out=rb_sb[:], in_=rbh[:])

        # ---- projections ----
        with tc.tile_pool(name="proj_ps", bufs=4, space="PSUM") as pps:
            # q: [64, NQ] (2 col-group replicas), ACT eviction adds bias
            for nb in range(NQ // IB):
                ps = pps.tile([2 * CQ, IB], f32, name="qps", tag="pps")
                for g in range(2):
                    for k in range(2):
                        nc.tensor.matmul(
                            ps[g * CQ:(g + 1) * CQ, :],
                            lhsT=wq_sb[:, k, :],
                            rhs=xb_sb[k][:, nb * IB:(nb + 1) * IB],
                            start=(k == 0), stop=(k == 1),
                            tile_position=(0, g * CQ))
                nc.scalar.activation(
                    out=qrep[:, nb * IB:(nb + 1) * IB], in_=ps[:],
                    func=FT.Identity, bias=bq_sb[:])
            # k: [64, N], no bias (cancels in softmax); DVE eviction
            for nb in range(N // IB):
                ps = pps.tile([2 * CQ, IB], f32, name="kps", tag="pps")
                for g in range(2):
                    for k in range(2):
                        nc.tensor.matmul(
                            ps[g * CQ:(g + 1) * CQ, :],
                            lhsT=wk_sb[:, k, :],
                            rhs=xb_sb[k][:, nb * IB:(nb + 1) * IB],
                            start=(k == 0), stop=(k == 1),
                            tile_position=(0, g * CQ))
                nc.vector.tensor_copy(
                    out=krep[:, nb * IB:(nb + 1) * IB], in_=ps[:])
            # V^T = x^T Wv^T in [j, c] layout (gamma folded into Wv on host)
            for jc in range(NJC):
                ps = pps.tile([P, C], f32, name="vps", tag="pps")
                for k in range(2):
                    nc.tensor.matmul(
                        ps[:],
                        lhsT=xb_sb[k][:, jc * P:(jc + 1) * P],
                        rhs=wv_sb[:, k, :],
                        start=(k == 0), stop=(k == 1))
                nc.vector.tensor_copy(out=vt_sb[:, jc, :], in_=ps[:])

        # ---- attention main loop ----
        sp_ps = ctx.enter_context(tc.tile_pool(name="sp_ps", bufs=3, space="PSUM"))
        out_ps = ctx.enter_context(tc.tile_pool(name="out_ps", bufs=1, space="PSUM"))
        pp_pool = ctx.enter_context(tc.tile_pool(name="ppp", bufs=4))
        acc_pool = ctx.enter_context(tc.tile_pool(name="accp", bufs=4))
        osb_pool = ctx.enter_context(tc.tile_pool(name="osbp", bufs=2))
        ysb_pool = ctx.enter_context(tc.tile_pool(name="ysbp", bufs=4))
        rec_pool = ctx.enter_context(tc.tile_pool(name="recp", bufs=2))

        NIB = NQ // IB
        NSRT = NIB * NSR   # total rounds

        def emit_s(sr):
            """S^T matmul pair for round sr: two concurrent 32-row-band MMs."""
            ib, lsr = divmod(sr, NSR)
            i0 = ib * IB
            sp = sp_ps.tile([P, 2, IB], f32, name="sp")
            for g in range(2):
                jc = 2 * lsr + g
                bb = g * CQ
                nc.tensor.matmul(
                    sp[:, g, :],
                    lhsT=krep[bb:bb + CQ, jc * P:(jc + 1) * P],
                    rhs=qrep[bb:bb + CQ, i0:i0 + IB],
                    start=True, stop=True,
                    tile_position=(bb, 0))
            return sp

        # software pipeline: S for round sr+2 is emitted right after round
        # sr's out matmuls, so its LDWEIGHTS prefetch during them and the
        # PE never waits on the exp stream.
        sps = {0: emit_s(0), 1: emit_s(1)}
        outp = None
        accs = None
        pending = []
        for sr in range(NSRT):
            ib, lsr = divmod(sr, NSR)
            i0 = ib * IB
            if lsr == 0:
                outp = out_ps.tile([P, 2, IB], f32, name="outp")
                accs = [None, None]
            ppt = pp_pool.tile([P, 2, IB], bf16, name="pp")
            nc.scalar.activation(out=ppt[:], in_=sps.pop(sr)[:], func=FT.Exp)
            ch = lsr // (NSR // 2)   # rowsum chain A: rounds 0-7, B: 8-14
            if lsr == NSR - 1:
                pass  # last round's pp reduces directly in the rowsum MMs
            elif lsr % (NSR // 2) == 0:
                accs[ch] = acc_pool.tile([P, 2, IB], bf16, name=f"acc{ch}")
                nc.vector.tensor_copy(out=accs[ch][:], in_=ppt[:])
            else:
                nc.vector.tensor_add(accs[ch][:], accs[ch][:], ppt[:])
            # final round: chains B and this round's pp reduce BEFORE the out
            # matmuls (same exp gate), so recip/broadcast start ~1us earlier
            # and the next i-block's S rounds never wait on the epi psum slot
            if lsr == NSR - 1:
                rs = accs[2][0:1, 0, :]
                for g in range(2):
                    nc.tensor.matmul(rs, lhsT=ones_col[:],
                                     rhs=accs[1][:, g, :],
                                     start=False, stop=False)
                for g in range(2):
                    nc.tensor.matmul(rs, lhsT=ones_col[:],
                                     rhs=ppt[:, g, :],
                                     start=False, stop=(g == 1))
            # S pairs for rounds sr+2 and sr+3 are emitted together (one
            # tile-mode run, LDWs pipeline) before this round's out matmuls;
            # with sp bufs=3 their psum WAR is already satisfied, so the exp
            # stream never waits on S production.
            if lsr % 2 == 0:
                for dd in (2, 3):
                    if sr + dd < NSRT:
                        sps[sr + dd] = emit_s(sr + dd)
            for g in range(2):
                jc = 2 * lsr + g
                for cc in range(2):
                    nc.tensor.matmul(
                        outp[:, cc, :],
                        lhsT=vt_sb[:, jc, cc * P:(cc + 1) * P],
                        rhs=ppt[:, g, :],
                        start=(lsr == 0 and g == 0),
                        stop=(lsr == NSR - 1 and g == 1))
            # rowsum chain A (rounds 0-7) reduces early so the epilogue's
            # recip chain at the i-block boundary only waits on chain B.
            if lsr == NSR - 3:
                # epilogue psum (rowsum + recip broadcast) lives in an sp
                # pool slot: keeps total PSUM within 8 banks.
                epi = sp_ps.tile([P, 2, IB], f32, name="sp")
                rs = epi[0:1, 0, :]
                for g in range(2):
                    nc.tensor.matmul(rs, lhsT=ones_col[:],
                                     rhs=accs[0][:, g, :],
                                     start=(g == 0), stop=False)
                accs.append(epi)
            if lsr < NSR - 1:
                continue
            # ---- epilogue for this i-block ----
            epi = accs[2]
            recip = rec_pool.tile([1, IB], f32, name="recip")
            nc.vector.reciprocal_approx_fast(out=recip[:], in_=epi[0:1, 0, :])
            osbs = []
            for cc in range(2):   # evict psum so next i-block's matmuls proceed
                osb = osb_pool.tile([P, IB], f32, name=f"osb{cc}")
                nc.vector.tensor_copy(out=osb[:], in_=outp[:, cc, :])
                osbs.append(osb)
            bc = epi[:, 1, :]
            nc.tensor.matmul(bc, lhsT=ones_row[:], rhs=recip[:],
                             start=True, stop=True)
            for cc in range(2):
                tmp = ysb_pool.tile([P, IB], f32, name="tmp")
                nc.vector.tensor_mul(tmp[:], osbs[cc][:], bc)
                ysb = ysb_pool.tile([P, IB], f32, name="ysb")
                nc.vector.scalar_tensor_tensor(
                    out=ysb[:], in0=tmp[:], scalar=rb_sb[:, cc:cc + 1],
                    in1=xb_sb[cc][:, i0:i0 + IB],
                    op0=ALU.add, op1=ALU.add)
                nc.sync.dma_start(out=y[cc * P:(cc + 1) * P, i0:i0 + IB],
                                  in_=ysb[:])

    nc.compile()
    return nc


def _get_nc():
    if "nc" not in _CACHE:
        _CACHE["nc"] = _build()
    return _CACHE["nc"]


def _fold_bn(w, b, g, beta, mean, var):
    s = g / np.sqrt(var + EPS)
    return w * s[:, None], b * s + beta - mean * s


def _in_maps(inputs):
    gx = np.asarray(inputs["x"], np.float32)
    gamma = float(np.asarray(inputs["gamma"]).reshape(-1)[0])
    wq, bq_ = _fold_bn(*[np.asarray(inputs[k], np.float32) for k in
                         ("q_w", "q_b", "q_g", "q_beta", "q_mean", "q_var")])
    wk, _bk = _fold_bn(*[np.asarray(inputs[k], np.float32) for k in
                         ("k_w", "k_b", "k_g", "k_beta", "k_mean", "k_var")])
    wv, bv_ = _fold_bn(*[np.asarray(inputs[k], np.float32) for k in
                         ("v_w", "v_b", "v_g", "v_beta", "v_mean", "v_var")])
    wqT = np.ascontiguousarray(wq.T.astype(BF))
    wkT = np.ascontiguousarray(wk.T.astype(BF))
    wvT = np.ascontiguousarray((gamma * wv).T.astype(BF))
    bq2 = np.ascontiguousarray(np.tile(bq_.reshape(CQ, 1), (2, 1)))
    rbh = np.ascontiguousarray((gamma * bv_).reshape(2, P).T)
    maps = []
    for core in range(NCORES):
        b, h = divmod(core, 2)
        xf = gx[b].reshape(C, N).astype(BF)
        if h == 1:  # own query-half columns first; key order is irrelevant
            xf = np.concatenate([xf[:, NQ:], xf[:, :NQ]], axis=1)
        maps.append({
            "xbh": np.ascontiguousarray(xf),
            "wqT": wqT, "wkT": wkT, "wvT": wvT,
            "bq2": bq2, "rbh": rbh,
        })
    return maps


def _gather(results):
    out = np.empty((B, C, N), np.float32)
    for core in range(NCORES):
        b, h = divmod(core, 2)
        out[b][:, h * NQ:(h + 1) * NQ] = results[core]["y"]
    return out.reshape(B, C, H, W)


def _run(inputs, **kw):
    nc = _get_nc()
    res = run_bass_kernel_spmd(nc, _in_maps(inputs),
                               core_ids=list(range(NCORES)), **kw)
    return res


def kernel(**inputs) -> np.ndarray:
    return _gather(_run(inputs).results)


# revision 49
# speedup vs baseline: 1.0208x; 1.0208x over previous
"""Trainium2 Bass kernel for nn_AttentionLayer (self-attention over 64x64 images).

Computation (per batch image b):
    xf = x[b].reshape(C, N)                     # C=256, N=4096
    q = BN(Wq @ xf + bq)   -> [32, N]
    k = BN(Wk @ xf + bk)   -> [32, N]
    v = BN(Wv @ xf + bv)   -> [256, N]
    attn = softmax_j(q^T k) -> [N, N]
    out = v @ attn^T        -> [256, N]
    y = gamma * out + xf

Sharding: 8 cores = 4 batches x 2 query-row halves; no collectives.

Host-side algebra folds (all exact):
  - BN folded into weights/bias.
  - k bias drops entirely (adds a per-query constant to every logit row ->
    cancels in softmax); q bias kept (its logit term varies over keys).
  - v bias passes through softmax unchanged (rows sum to 1), so
    y = (gamma*vhat @ P)*recip + (x + gamma*bv); gamma folds into Wv and the
    residual bias rb = gamma*bv is added per-channel in the epilogue.
  - Each core's xbh has its own query-half columns FIRST (keys are
    order-invariant), so the query slice is a view of the key tensor and the
    bf16 x tile doubles as the residual.

Device (per core, all matmuls bf16 in / f32 psum):
  - q/k proj: 2 col-group replica matmuls -> [64, N*]; q evicted via ACT
    Identity+bias, k via DVE copy; DVE copy replicates to partitions 64-127
    so S^T rounds can run 4 concurrent row-band (32x128-tile) matmuls.
  - V^T[j, c] = x^T Wv^T computed directly in [j, c] layout.
  - Main loop over 4 i-blocks x 16 rounds of 2 key-chunks: S^T row-band
    matmul pair -> exp on ScalarE (no max subtraction; |S|<60 fits f32/bf16)
    -> P^T bf16 -> out psum accumulation + bf16 rowsum chains on DVE.
    S matmuls for round pairs are emitted adjacently for 4-way PE tile
    concurrency and fewer tile-mode switches.
  - Epilogue per i-block: outp evicted to SBUF (frees psum banks), rowsum
    via ones-matmuls, reciprocal on DVE, PE rank-1 broadcast, fused
    (out*recip + rb) + x on DVE, DMA out.
"""

import numpy as np
from contextlib import ExitStack

import ml_dtypes
import concourse.bass as bass
import concourse.mybir as mybir
import concourse.tile as tile
from concourse import bacc
from concourse.bass_utils import run_bass_kernel_spmd

B, C, H, W = 4, 256, 64, 64
N = H * W            # 4096 tokens per image
CQ = C // 8          # 32 q/k channels
NQ = N // 2          # 2048 query tokens per core
EPS = 1e-5
P = 128
IB = 512             # i-block (psum bank of f32)
NJC = N // P         # 32 j-chunks
NSR = NJC // 2       # 16 rounds per i-block (2 j-chunks each)
NCORES = 8

f32 = mybir.dt.float32
bf16 = mybir.dt.bfloat16
FT = mybir.ActivationFunctionType
ALU = mybir.AluOpType
BF = ml_dtypes.bfloat16

_CACHE = {}


def _build():
    nc = bacc.Bacc("TRN2", target_bir_lowering=False, debug=False,
                   num_devices=NCORES)
    xbh = nc.dram_tensor("xbh", [C, N], bf16, kind="ExternalInput").ap()
    wqT = nc.dram_tensor("wqT", [C, CQ], bf16, kind="ExternalInput").ap()
    wkT = nc.dram_tensor("wkT", [C, CQ], bf16, kind="ExternalInput").ap()
    wvT = nc.dram_tensor("wvT", [C, C], bf16, kind="ExternalInput").ap()
    bq2 = nc.dram_tensor("bq2", [2 * CQ, 1], f32, kind="ExternalInput").ap()
    rbh = nc.dram_tensor("rbh", [P, 2], f32, kind="ExternalInput").ap()
    y = nc.dram_tensor("y", [C, NQ], f32, kind="ExternalOutput").ap()

    with tile.TileContext(nc) as tc, ExitStack() as ctx:
        const = ctx.enter_context(tc.tile_pool(name="const", bufs=1))
        ones_col = const.tile([P, 1], bf16)
        nc.vector.memset(ones_col[:], 1.0)
        ones_row = const.tile([1, P], f32)
        nc.vector.memset(ones_row[:], 1.0)

        wq_sb = const.tile([P, 2, CQ], bf16)
        wk_sb = const.tile([P, 2, CQ], bf16)
        wv_sb = const.tile([P, 2, C], bf16)
        bq_sb = const.tile([2 * CQ, 1], f32)
        rb_sb = const.tile([P, 2], f32)

        garb = const.tile([P, 64], bf16)
        nc.vector.memset(garb[:], 0.5)

        xpool = ctx.enter_context(tc.tile_pool(name="x", bufs=1))
        xb_sb = [xpool.tile([P, N], bf16, name=f"xbsb{cc}") for cc in range(2)]

        # PE warmup during the input-DMA window: ~50 dependency-free tiny
        # matmuls keep the PE busy past the HAM activity window so the
        # projection phase starts at the full 2.4 GHz clock.
        with tc.tile_pool(name="warm_ps", bufs=1, space="PSUM") as wps:
            wtile = wps.tile([1, 64], f32, name="warm")
            for _ in range(72):
                nc.tensor.matmul(wtile[:], lhsT=ones_col[:], rhs=garb[:],
                                 start=True, stop=True)

        qkv = ctx.enter_context(tc.tile_pool(name="qkv", bufs=1))
        qrep = qkv.tile([2 * CQ, NQ], bf16)  # 2 row-band replicas of q
        krep = qkv.tile([2 * CQ, N], bf16)   # 2 row-band replicas of k
        vt_sb = qkv.tile([P, NJC, C], bf16)  # V^T as [j-in-chunk, jc, c]

        # ---- input DMA (query-half columns of xbh come first) ----
        nc.sync.dma_start(out=wq_sb[:], in_=wqT.rearrange("(k p) m -> p k m", k=2))
        nc.sync.dma_start(out=bq_sb[:], in_=bq2[:])
        for cc in range(2):   # first 512 columns alone so q proj starts early
            nc.sync.dma_start(out=xb_sb[cc][:, 0:IB],
                              in_=xbh[cc * P:(cc + 1) * P, 0:IB])
        for cc in range(2):
            nc.sync.dma_start(out=xb_sb[cc][:, IB:1024],
                              in_=xbh[cc * P:(cc + 1) * P, IB:1024])
        for cc in range(2):
            nc.sync.dma_start(out=xb_sb[cc][:, 1024:2048],
                              in_=xbh[cc * P:(cc + 1) * P, 1024:2048])
        nc.sync.dma_start(out=wk_sb[:],
                          in_=wkT.rearrange("(k p) m -> p k m", k=2))
        # second half of the keys + v-path weights issue from the Activation
        # queue (also a HWDGE on TRN2) in parallel with the Sync queue's
        # stream: each dma_start costs ~0.6us of issue time on its queue.
        for cc in range(2):
            nc.scalar.dma_start(out=xb_sb[cc][:, 2048:N],
                                in_=xbh[cc * P:(cc + 1) * P, 2048:N])
        nc.scalar.dma_start(out=wv_sb[:],
                            in_=wvT.rearrange("(k p) m -> p k m", k=2))
        nc.sync.dma_start(out=rb_sb[:], in_=rbh[:])

        # ---- projections ----
        with tc.tile_pool(name="proj_ps", bufs=4, space="PSUM") as pps:
            # q: [64, NQ] (2 col-group replicas), ACT eviction adds bias
            for nb in range(NQ // IB):
                ps = pps.tile([2 * CQ, IB], f32, name="qps", tag="pps")
                for g in range(2):
                    for k in range(2):
                        nc.tensor.matmul(
                            ps[g * CQ:(g + 1) * CQ, :],
                            lhsT=wq_sb[:, k, :],
                            rhs=xb_sb[k][:, nb * IB:(nb + 1) * IB],
                            start=(k == 0), stop=(k == 1),
                            tile_position=(0, g * CQ))
                nc.scalar.activation(
                    out=qrep[:, nb * IB:(nb + 1) * IB], in_=ps[:],
                    func=FT.Identity, bias=bq_sb[:])
            # k: [64, N], no bias (cancels in softmax); DVE eviction
            for nb in range(N // IB):
                ps = pps.tile([2 * CQ, IB], f32, name="kps", tag="pps")
                for g in range(2):
                    for k in range(2):
                        nc.tensor.matmul(
                            ps[g * CQ:(g + 1) * CQ, :],
                            lhsT=wk_sb[:, k, :],
                            rhs=xb_sb[k][:, nb * IB:(nb + 1) * IB],
                            start=(k == 0), stop=(k == 1),
                            tile_position=(0, g * CQ))
                nc.vector.tensor_copy(
                    out=krep[:, nb * IB:(nb + 1) * IB], in_=ps[:])
            # V^T = x^T Wv^T in [j, c] layout (gamma folded into Wv on host)
            for jc in range(NJC):
                ps = pps.tile([P, C], f32, name="vps", tag="pps")
                for k in range(2):
                    nc.tensor.matmul(
                        ps[:],
                        lhsT=xb_sb[k][:, jc * P:(jc + 1) * P],
                        rhs=wv_sb[:, k, :],
                        start=(k == 0), stop=(k == 1))
                nc.vector.tensor_copy(out=vt_sb[:, jc, :], in_=ps[:])

        # ---- attention main loop ----
        sp_ps = ctx.enter_context(tc.tile_pool(name="sp_ps", bufs=3, space="PSUM"))
        out_ps = ctx.enter_context(tc.tile_pool(name="out_ps", bufs=1, space="PSUM"))
        pp_pool = ctx.enter_context(tc.tile_pool(name="ppp", bufs=6))
        acc_pool = ctx.enter_context(tc.tile_pool(name="accp", bufs=4))
        osb_pool = ctx.enter_context(tc.tile_pool(name="osbp", bufs=2))
        ysb_pool = ctx.enter_context(tc.tile_pool(name="ysbp", bufs=4))
        rec_pool = ctx.enter_context(tc.tile_pool(name="recp", bufs=2))

        NIB = NQ // IB
        NSRT = NIB * NSR   # total rounds

        def emit_s(sr):
            """S^T matmul pair for round sr: two concurrent 32-row-band MMs."""
            ib, lsr = divmod(sr, NSR)
            i0 = ib * IB
            sp = sp_ps.tile([P, 2, IB], f32, name="sp")
            for g in range(2):
                jc = 2 * lsr + g
                bb = g * CQ
                nc.tensor.matmul(
                    sp[:, g, :],
                    lhsT=krep[bb:bb + CQ, jc * P:(jc + 1) * P],
                    rhs=qrep[bb:bb + CQ, i0:i0 + IB],
                    start=True, stop=True,
                    tile_position=(bb, 0))
            return sp

        # software pipeline: S for round sr+2 is emitted right after round
        # sr's out matmuls, so its LDWEIGHTS prefetch during them and the
        # PE never waits on the exp stream.
        sps = {0: emit_s(0), 1: emit_s(1)}
        outp = None
        accs = None
        pending = []
        for sr in range(NSRT):
            ib, lsr = divmod(sr, NSR)
            i0 = ib * IB
            if lsr == 0:
                outp = out_ps.tile([P, 2, IB], f32, name="outp")
                accs = [None, None]
            ppt = pp_pool.tile([P, 2, IB], bf16, name="pp")
            nc.scalar.activation(out=ppt[:], in_=sps.pop(sr)[:], func=FT.Exp)
            ch = lsr // (NSR // 2)   # rowsum chain A: rounds 0-7, B: 8-15
            if lsr % (NSR // 2) == 0:
                accs[ch] = acc_pool.tile([P, 2, IB], bf16, name=f"acc{ch}")
                nc.vector.tensor_copy(out=accs[ch][:], in_=ppt[:])
            else:
                nc.vector.tensor_add(accs[ch][:], accs[ch][:], ppt[:])
            # S pairs for rounds sr+2 and sr+3 are emitted together (one
            # tile-mode run, LDWs pipeline) before this round's out matmuls;
            # with sp bufs=3 their psum WAR is already satisfied, so the exp
            # stream never waits on S production.
            if lsr % 2 == 0:
                for dd in (2, 3):
                    if sr + dd < NSRT:
                        sps[sr + dd] = emit_s(sr + dd)
            for g in range(2):
                jc = 2 * lsr + g
                for cc in range(2):
                    nc.tensor.matmul(
                        outp[:, cc, :],
                        lhsT=vt_sb[:, jc, cc * P:(cc + 1) * P],
                        rhs=ppt[:, g, :],
                        start=(lsr == 0 and g == 0),
                        stop=(lsr == NSR - 1 and g == 1))
            # rowsum chain A (rounds 0-7) reduces early so the epilogue's
            # recip chain at the i-block boundary only waits on chain B.
            if lsr == NSR - 3:
                # epilogue psum (rowsum + recip broadcast) lives in an sp
                # pool slot: keeps total PSUM within 8 banks.
                epi = sp_ps.tile([P, 2, IB], f32, name="sp")
                rs = epi[0:1, 0, :]
                for g in range(2):
                    nc.tensor.matmul(rs, lhsT=ones_col[:],
                                     rhs=accs[0][:, g, :],
                                     start=(g == 0), stop=False)
                accs.append(epi)
            if lsr < NSR - 1:
                continue
            # ---- epilogue for this i-block ----
            osbs = []
            for cc in range(2):   # evict psum so next i-block's matmuls proceed
                osb = osb_pool.tile([P, IB], f32, name=f"osb{cc}")
                nc.vector.tensor_copy(out=osb[:], in_=outp[:, cc, :])
                osbs.append(osb)
            epi = accs[2]
            rs = epi[0:1, 0, :]
            for g in range(2):
                nc.tensor.matmul(rs, lhsT=ones_col[:],
                                 rhs=accs[1][:, g, :],
                                 start=False, stop=(g == 1))
            recip = rec_pool.tile([1, IB], f32, name="recip")
            nc.vector.reciprocal_approx_fast(out=recip[:], in_=rs)
            bc = epi[:, 1, :]
            nc.tensor.matmul(bc, lhsT=ones_row[:], rhs=recip[:],
                             start=True, stop=True)
            for cc in range(2):
                tmp = ysb_pool.tile([P, IB], f32, name="tmp")
                nc.vector.tensor_mul(tmp[:], osbs[cc][:], bc)
                ysb = ysb_pool.tile([P, IB], f32, name="ysb")
                nc.vector.scalar_tensor_tensor(
                    out=ysb[:], in0=tmp[:], scalar=rb_sb[:, cc:cc + 1],
                    in1=xb_sb[cc][:, i0:i0 + IB],
                    op0=ALU.add, op1=ALU.add)
                nc.sync.dma_start(out=y[cc * P:(cc + 1) * P, i0:i0 + IB],
                                  in_=ysb[:])

    nc.compile()
    return nc


def _get_nc():
    if "nc" not in _CACHE:
        _CACHE["nc"] = _build()
    return _CACHE["nc"]


def _fold_bn(w, b, g, beta, mean, var):
    s = g / np.sqrt(var + EPS)
    return w * s[:, None], b * s + beta - mean * s


def _in_maps(inputs):
    gx = np.asarray(inputs["x"], np.float32)
    gamma = float(np.asarray(inputs["gamma"]).reshape(-1)[0])
    wq, bq_ = _fold_bn(*[np.asarray(inputs[k], np.float32) for k in
                         ("q_w", "q_b", "q_g", "q_beta", "q_mean", "q_var")])
    wk, _bk = _fold_bn(*[np.asarray(inputs[k], np.float32) for k in
                         ("k_w", "k_b", "k_g", "k_beta", "k_mean", "k_var")])
    wv, bv_ = _fold_bn(*[np.asarray(inputs[k], np.float32) for k in
                         ("v_w", "v_b", "v_g", "v_beta", "v_mean", "v_var")])
    wqT = np.ascontiguousarray(wq.T.astype(BF))
    wkT = np.ascontiguousarray(wk.T.astype(BF))
    wvT = np.ascontiguousarray((gamma * wv).T.astype(BF))
    bq2 = np.ascontiguousarray(np.tile(bq_.reshape(CQ, 1), (2, 1)))
    rbh = np.ascontiguousarray((gamma * bv_).reshape(2, P).T)
    maps = []
    for core in range(NCORES):
        b, h = divmod(core, 2)
        xf = gx[b].reshape(C, N).astype(BF)
        if h == 1:  # own query-half columns first; key order is irrelevant
            xf = np.concatenate([xf[:, NQ:], xf[:, :NQ]], axis=1)
        maps.append({
            "xbh": np.ascontiguousarray(xf),
            "wqT": wqT, "wkT": wkT, "wvT": wvT,
            "bq2": bq2, "rbh": rbh,
        })
    return maps


def _gather(results):
    out = np.empty((B, C, N), np.float32)
    for core in range(NCORES):
        b, h = divmod(core, 2)
        out[b][:, h * NQ:(h + 1) * NQ] = results[core]["y"]
    return out.reshape(B, C, H, W)


def _run(inputs, **kw):
    nc = _get_nc()
    res = run_bass_kernel_spmd(nc, _in_maps(inputs),
                               core_ids=list(range(NCORES)), **kw)
    return res


def kernel(**inputs) -> np.ndarray:
    return _gather(_run(inputs).results)


# revision 51
# speedup vs baseline: 1.0261x; 1.0052x over previous
"""Trainium2 Bass kernel for nn_AttentionLayer (self-attention over 64x64 images).

Computation (per batch image b):
    xf = x[b].reshape(C, N)                     # C=256, N=4096
    q = BN(Wq @ xf + bq)   -> [32, N]
    k = BN(Wk @ xf + bk)   -> [32, N]
    v = BN(Wv @ xf + bv)   -> [256, N]
    attn = softmax_j(q^T k) -> [N, N]
    out = v @ attn^T        -> [256, N]
    y = gamma * out + xf

Sharding: 8 cores = 4 batches x 2 query-row halves; no collectives.

Host-side algebra folds (all exact):
  - BN folded into weights/bias.
  - k bias drops entirely (adds a per-query constant to every logit row ->
    cancels in softmax); q bias kept (its logit term varies over keys).
  - v bias passes through softmax unchanged (rows sum to 1), so
    y = (gamma*vhat @ P)*recip + (x + gamma*bv); gamma folds into Wv and the
    residual bias rb = gamma*bv is added per-channel in the epilogue.
  - Each core's xbh has its own query-half columns FIRST (keys are
    order-invariant), so the query slice is a view of the key tensor and the
    bf16 x tile doubles as the residual.

Device (per core, all matmuls bf16 in / f32 psum):
  - q/k proj: 2 col-group replica matmuls -> [64, N*]; q evicted via ACT
    Identity+bias, k via DVE copy; DVE copy replicates to partitions 64-127
    so S^T rounds can run 4 concurrent row-band (32x128-tile) matmuls.
  - V^T[j, c] = x^T Wv^T computed directly in [j, c] layout.
  - Main loop over 4 i-blocks x 16 rounds of 2 key-chunks: S^T row-band
    matmul pair -> exp on ScalarE (no max subtraction; |S|<60 fits f32/bf16)
    -> P^T bf16 -> out psum accumulation + bf16 rowsum chains on DVE.
    S matmuls for round pairs are emitted adjacently for 4-way PE tile
    concurrency and fewer tile-mode switches.
  - Epilogue per i-block: outp evicted to SBUF (frees psum banks), rowsum
    via ones-matmuls, reciprocal on DVE, PE rank-1 broadcast, fused
    (out*recip + rb) + x on DVE, DMA out.
"""

import numpy as np
from contextlib import ExitStack

import ml_dtypes
import concourse.bass as bass
import concourse.mybir as mybir
import concourse.tile as tile
from concourse import bacc
from concourse.bass_utils import run_bass_kernel_spmd

B, C, H, W = 4, 256, 64, 64
N = H * W            # 4096 tokens per image
CQ = C // 8          # 32 q/k channels
NQ = N // 2          # 2048 query tokens per core
EPS = 1e-5
P = 128
IB = 512             # i-block (psum bank of f32)
NJC = N // P         # 32 j-chunks
NSR = NJC // 2       # 16 rounds per i-block (2 j-chunks each)
NCORES = 8

f32 = mybir.dt.float32
bf16 = mybir.dt.bfloat16
FT = mybir.ActivationFunctionType
ALU = mybir.AluOpType
BF = ml_dtypes.bfloat16

_CACHE = {}


def _build():
    nc = bacc.Bacc("TRN2", target_bir_lowering=False, debug=False,
                   num_devices=NCORES)
    xbh = nc.dram_tensor("xbh", [C, N], bf16, kind="ExternalInput").ap()
    wqT = nc.dram_tensor("wqT", [C, CQ], bf16, kind="ExternalInput").ap()
    wkT = nc.dram_tensor("wkT", [C, CQ], bf16, kind="ExternalInput").ap()
    wvT = nc.dram_tensor("wvT", [C, C], bf16, kind="ExternalInput").ap()
    bq2 = nc.dram_tensor("bq2", [2 * CQ, 1], f32, kind="ExternalInput").ap()
    rbh = nc.dram_tensor("rbh", [P, 2], f32, kind="ExternalInput").ap()
    y = nc.dram_tensor("y", [C, NQ], bf16, kind="ExternalOutput").ap()

    with tile.TileContext(nc) as tc, ExitStack() as ctx:
        const = ctx.enter_context(tc.tile_pool(name="const", bufs=1))
        ones_col = const.tile([P, 1], bf16)
        nc.vector.memset(ones_col[:], 1.0)
        ones_row = const.tile([1, P], f32)
        nc.vector.memset(ones_row[:], 1.0)

        wq_sb = const.tile([P, 2, CQ], bf16)
        wk_sb = const.tile([P, 2, CQ], bf16)
        wv_sb = const.tile([P, 2, C], bf16)
        bq_sb = const.tile([2 * CQ, 1], f32)
        rb_sb = const.tile([P, 2], f32)

        garb = const.tile([P, 64], bf16)
        nc.vector.memset(garb[:], 0.5)

        xpool = ctx.enter_context(tc.tile_pool(name="x", bufs=1))
        xb_sb = [xpool.tile([P, N], bf16, name=f"xbsb{cc}") for cc in range(2)]

        # PE warmup during the input-DMA window: ~50 dependency-free tiny
        # matmuls keep the PE busy past the HAM activity window so the
        # projection phase starts at the full 2.4 GHz clock.
        with tc.tile_pool(name="warm_ps", bufs=1, space="PSUM") as wps:
            wtile = wps.tile([1, 64], f32, name="warm")
            for _ in range(72):
                nc.tensor.matmul(wtile[:], lhsT=ones_col[:], rhs=garb[:],
                                 start=True, stop=True)

        qkv = ctx.enter_context(tc.tile_pool(name="qkv", bufs=1))
        qrep = qkv.tile([2 * CQ, NQ], bf16)  # 2 row-band replicas of q
        krep = qkv.tile([2 * CQ, N], bf16)   # 2 row-band replicas of k
        vt_sb = qkv.tile([P, NJC, C], bf16)  # V^T as [j-in-chunk, jc, c]

        # ---- input DMA (query-half columns of xbh come first) ----
        nc.sync.dma_start(out=wq_sb[:], in_=wqT.rearrange("(k p) m -> p k m", k=2))
        nc.sync.dma_start(out=bq_sb[:], in_=bq2[:])
        for cc in range(2):   # first 512 columns alone so q proj starts early
            nc.sync.dma_start(out=xb_sb[cc][:, 0:IB],
                              in_=xbh[cc * P:(cc + 1) * P, 0:IB])
        for cc in range(2):
            nc.sync.dma_start(out=xb_sb[cc][:, IB:1024],
                              in_=xbh[cc * P:(cc + 1) * P, IB:1024])
        for nb in range(1, N // 1024):
            for cc in range(2):
                nc.sync.dma_start(
                    out=xb_sb[cc][:, nb * 1024:(nb + 1) * 1024],
                    in_=xbh[cc * P:(cc + 1) * P, nb * 1024:(nb + 1) * 1024])
            if nb == 1:
                nc.sync.dma_start(out=wk_sb[:],
                                  in_=wkT.rearrange("(k p) m -> p k m", k=2))
            if nb == 2:
                nc.sync.dma_start(out=wv_sb[:],
                                  in_=wvT.rearrange("(k p) m -> p k m", k=2))
        nc.sync.dma_start(out=rb_sb[:], in_=rbh[:])

        # ---- projections ----
        with tc.tile_pool(name="proj_ps", bufs=4, space="PSUM") as pps:
            # q: [64, NQ] (2 col-group replicas), ACT eviction adds bias
            for nb in range(NQ // IB):
                ps = pps.tile([2 * CQ, IB], f32, name="qps", tag="pps")
                for g in range(2):
                    for k in range(2):
                        nc.tensor.matmul(
                            ps[g * CQ:(g + 1) * CQ, :],
                            lhsT=wq_sb[:, k, :],
                            rhs=xb_sb[k][:, nb * IB:(nb + 1) * IB],
                            start=(k == 0), stop=(k == 1),
                            tile_position=(0, g * CQ))
                nc.scalar.activation(
                    out=qrep[:, nb * IB:(nb + 1) * IB], in_=ps[:],
                    func=FT.Identity, bias=bq_sb[:])
            # k: [64, N], no bias (cancels in softmax); DVE eviction
            for nb in range(N // IB):
                ps = pps.tile([2 * CQ, IB], f32, name="kps", tag="pps")
                for g in range(2):
                    for k in range(2):
                        nc.tensor.matmul(
                            ps[g * CQ:(g + 1) * CQ, :],
                            lhsT=wk_sb[:, k, :],
                            rhs=xb_sb[k][:, nb * IB:(nb + 1) * IB],
                            start=(k == 0), stop=(k == 1),
                            tile_position=(0, g * CQ))
                nc.vector.tensor_copy(
                    out=krep[:, nb * IB:(nb + 1) * IB], in_=ps[:])
            # V^T = x^T Wv^T in [j, c] layout (gamma folded into Wv on host)
            for jc in range(NJC):
                ps = pps.tile([P, C], f32, name="vps", tag="pps")
                for k in range(2):
                    nc.tensor.matmul(
                        ps[:],
                        lhsT=xb_sb[k][:, jc * P:(jc + 1) * P],
                        rhs=wv_sb[:, k, :],
                        start=(k == 0), stop=(k == 1))
                nc.vector.tensor_copy(out=vt_sb[:, jc, :], in_=ps[:])

        # ---- attention main loop ----
        sp_ps = ctx.enter_context(tc.tile_pool(name="sp_ps", bufs=3, space="PSUM"))
        out_ps = ctx.enter_context(tc.tile_pool(name="out_ps", bufs=1, space="PSUM"))
        pp_pool = ctx.enter_context(tc.tile_pool(name="ppp", bufs=4))
        acc_pool = ctx.enter_context(tc.tile_pool(name="accp", bufs=4))
        osb_pool = ctx.enter_context(tc.tile_pool(name="osbp", bufs=2))
        ysb_pool = ctx.enter_context(tc.tile_pool(name="ysbp", bufs=4))
        rec_pool = ctx.enter_context(tc.tile_pool(name="recp", bufs=2))

        NIB = NQ // IB
        NSRT = NIB * NSR   # total rounds

        def emit_s(sr):
            """S^T matmul pair for round sr: two concurrent 32-row-band MMs."""
            ib, lsr = divmod(sr, NSR)
            i0 = ib * IB
            sp = sp_ps.tile([P, 2, IB], f32, name="sp")
            for g in range(2):
                jc = 2 * lsr + g
                bb = g * CQ
                nc.tensor.matmul(
                    sp[:, g, :],
                    lhsT=krep[bb:bb + CQ, jc * P:(jc + 1) * P],
                    rhs=qrep[bb:bb + CQ, i0:i0 + IB],
                    start=True, stop=True,
                    tile_position=(bb, 0))
            return sp

        # software pipeline: S for round sr+2 is emitted right after round
        # sr's out matmuls, so its LDWEIGHTS prefetch during them and the
        # PE never waits on the exp stream.
        sps = {0: emit_s(0), 1: emit_s(1)}
        outp = None
        accs = None
        pending = []
        for sr in range(NSRT):
            ib, lsr = divmod(sr, NSR)
            i0 = ib * IB
            if lsr == 0:
                outp = out_ps.tile([P, 2, IB], f32, name="outp")
                accs = [None, None]
            ppt = pp_pool.tile([P, 2, IB], bf16, name="pp")
            nc.scalar.activation(out=ppt[:], in_=sps.pop(sr)[:], func=FT.Exp)
            ch = lsr // (NSR // 2)   # rowsum chain A: rounds 0-7, B: 8-15
            if lsr % (NSR // 2) == 0:
                accs[ch] = acc_pool.tile([P, 2, IB], bf16, name=f"acc{ch}")
                nc.vector.tensor_copy(out=accs[ch][:], in_=ppt[:])
            else:
                nc.vector.tensor_add(accs[ch][:], accs[ch][:], ppt[:])
            # S pairs for rounds sr+2 and sr+3 are emitted together (one
            # tile-mode run, LDWs pipeline) before this round's out matmuls;
            # with sp bufs=3 their psum WAR is already satisfied, so the exp
            # stream never waits on S production.
            if lsr % 2 == 0:
                for dd in (2, 3):
                    if sr + dd < NSRT:
                        sps[sr + dd] = emit_s(sr + dd)
            for g in range(2):
                jc = 2 * lsr + g
                for cc in range(2):
                    nc.tensor.matmul(
                        outp[:, cc, :],
                        lhsT=vt_sb[:, jc, cc * P:(cc + 1) * P],
                        rhs=ppt[:, g, :],
                        start=(lsr == 0 and g == 0),
                        stop=(lsr == NSR - 1 and g == 1))
            # rowsum chain A (rounds 0-7) reduces early so the epilogue's
            # recip chain at the i-block boundary only waits on chain B.
            if lsr == NSR - 3:
                # epilogue psum (rowsum + recip broadcast) lives in an sp
                # pool slot: keeps total PSUM within 8 banks.
                epi = sp_ps.tile([P, 2, IB], f32, name="sp")
                rs = epi[0:1, 0, :]
                for g in range(2):
                    nc.tensor.matmul(rs, lhsT=ones_col[:],
                                     rhs=accs[0][:, g, :],
                                     start=(g == 0), stop=False)
                accs.append(epi)
            if lsr < NSR - 1:
                continue
            # ---- epilogue for this i-block ----
            osbs = []
            for cc in range(2):   # evict psum so next i-block's matmuls proceed
                osb = osb_pool.tile([P, IB], f32, name=f"osb{cc}")
                nc.vector.tensor_copy(out=osb[:], in_=outp[:, cc, :])
                osbs.append(osb)
            epi = accs[2]
            rs = epi[0:1, 0, :]
            for g in range(2):
                nc.tensor.matmul(rs, lhsT=ones_col[:],
                                 rhs=accs[1][:, g, :],
                                 start=False, stop=(g == 1))
            recip = rec_pool.tile([1, IB], f32, name="recip")
            nc.vector.reciprocal_approx_fast(out=recip[:], in_=rs)
            bc = epi[:, 1, :]
            nc.tensor.matmul(bc, lhsT=ones_row[:], rhs=recip[:],
                             start=True, stop=True)
            for cc in range(2):
                tmp = ysb_pool.tile([P, IB], f32, name="tmp")
                nc.vector.tensor_mul(tmp[:], osbs[cc][:], bc)
                ysb = ysb_pool.tile([P, IB], bf16, name="ysb")
                nc.vector.scalar_tensor_tensor(
                    out=ysb[:], in0=tmp[:], scalar=rb_sb[:, cc:cc + 1],
                    in1=xb_sb[cc][:, i0:i0 + IB],
                    op0=ALU.add, op1=ALU.add)
                nc.sync.dma_start(out=y[cc * P:(cc + 1) * P, i0:i0 + IB],
                                  in_=ysb[:])

    nc.compile()
    return nc


def _get_nc():
    if "nc" not in _CACHE:
        _CACHE["nc"] = _build()
    return _CACHE["nc"]


def _fold_bn(w, b, g, beta, mean, var):
    s = g / np.sqrt(var + EPS)
    return w * s[:, None], b * s + beta - mean * s


def _in_maps(inputs):
    gx = np.asarray(inputs["x"], np.float32)
    gamma = float(np.asarray(inputs["gamma"]).reshape(-1)[0])
    wq, bq_ = _fold_bn(*[np.asarray(inputs[k], np.float32) for k in
                         ("q_w", "q_b", "q_g", "q_beta", "q_mean", "q_var")])
    wk, _bk = _fold_bn(*[np.asarray(inputs[k], np.float32) for k in
                         ("k_w", "k_b", "k_g", "k_beta", "k_mean", "k_var")])
    wv, bv_ = _fold_bn(*[np.asarray(inputs[k], np.float32) for k in
                         ("v_w", "v_b", "v_g", "v_beta", "v_mean", "v_var")])
    wqT = np.ascontiguousarray(wq.T.astype(BF))
    wkT = np.ascontiguousarray(wk.T.astype(BF))
    wvT = np.ascontiguousarray((gamma * wv).T.astype(BF))
    bq2 = np.ascontiguousarray(np.tile(bq_.reshape(CQ, 1), (2, 1)))
    rbh = np.ascontiguousarray((gamma * bv_).reshape(2, P).T)
    maps = []
    for core in range(NCORES):
        b, h = divmod(core, 2)
        xf = gx[b].reshape(C, N).astype(BF)
        if h == 1:  # own query-half columns first; key order is irrelevant
            xf = np.concatenate([xf[:, NQ:], xf[:, :NQ]], axis=1)
        maps.append({
            "xbh": np.ascontiguousarray(xf),
            "wqT": wqT, "wkT": wkT, "wvT": wvT,
            "bq2": bq2, "rbh": rbh,
        })
    return maps


def _gather(results):
    out = np.empty((B, C, N), np.float32)
    for core in range(NCORES):
        b, h = divmod(core, 2)
        out[b][:, h * NQ:(h + 1) * NQ] = results[core]["y"]
    return out.reshape(B, C, H, W)


def _run(inputs, **kw):
    nc = _get_nc()
    res = run_bass_kernel_spmd(nc, _in_maps(inputs),
                               core_ids=list(range(NCORES)), **kw)
    return res


def kernel(**inputs) -> np.ndarray:
    return _gather(_run(inputs).results)
